# revision 18
# baseline (speedup 1.0000x reference)
"""AdaptiveNodeCollapse Trainium2 kernel (8 NeuronCores, batch-sharded).

Reference semantics: normalize clusters along D, compute per-batch cosine
similarity, OR (sim > 0.9) over the batch, then a sequential merge scan over
upper-triangle pairs with the flags fixed up front.

Device work per core (8 of the 64 batches):
  - pass-through copy clusters -> out (the merge is the identity whenever no
    pair crosses the threshold, which is the memory-bound hot path),
  - normalized Gram matrices via bf16 TensorE matmuls, max-reduced over the
    local batch -> per-core maxsim output.
Host: OR/max the 8 maxsim outputs. Only if some pair is near/above the
threshold (never for randn-scale data) recompute flags exactly in f32 and
apply the reference merge scan on the pass-through output.

Layout notes: rows are loaded 2-per-partition (q = 2p+j) so DMA packets are
6KB contiguous; the bf16 xbar DMA-transpose then yields column blocks of
even q (0:128) and odd q (128:256), i.e. maxsim comes out row/col permuted
by [0,2,...,254,1,3,...,255] — undone on the host. The d-axis enumeration
the transpose produces is a bijection, which the matmul contraction is
invariant to.
"""

import numpy as np

import concourse.bass as bass
import concourse.mybir as mybir
import concourse.tile as tile
from concourse import bacc
from concourse.bass_utils import run_bass_kernel_spmd

B, Q, D = 64, 256, 768
NCORES = 8
BL = B // NCORES  # batches per core
KC2 = D // 256    # contraction chunks of 128 fp8-pairs (DoubleRow K=256)
THRESHOLD = 0.9
# Device sim is computed in fp8e4m3 (f32 accumulate); for unit vectors its
# error is < ~0.13, so any true-f32 sim > 0.9 shows up as device sim > 0.77,
# while randn-scale data (true max sim ~0.25) stays far below the trigger.
DEVICE_TRIGGER = 0.75
EPS = 1e-12

f32 = mybir.dt.float32
fp8 = mybir.dt.float8e4
u16 = mybir.dt.uint16

_nc_cache = None


def _build():
    nc = bacc.Bacc("TRN2", target_bir_lowering=False, debug=False,
                   num_devices=NCORES)
    cl = nc.dram_tensor("clusters", [BL, Q, D], f32, kind="ExternalInput").ap()
    out = nc.dram_tensor("out", [BL, Q, D], f32, kind="ExternalOutput").ap()
    msim = nc.dram_tensor("maxsim", [Q, Q], f32, kind="ExternalOutput").ap()

    with tile.TileContext(nc) as tc:
        with (
            tc.tile_pool(name="xp", bufs=BL // 2 + 1) as xp,
            tc.tile_pool(name="sqp", bufs=3) as sqp,
            tc.tile_pool(name="yp", bufs=BL // 2 + 1) as yp,
            tc.tile_pool(name="ytp", bufs=BL // 2 + 1) as ytp,
            tc.tile_pool(name="normp", bufs=2 * BL) as normp,
            tc.tile_pool(name="accp", bufs=1) as accp,
            tc.tile_pool(name="ps0", bufs=2, space="PSUM") as ps0p,
            tc.tile_pool(name="ps1", bufs=2, space="PSUM") as ps1p,
        ):
            acc0 = accp.tile([128, Q], f32, tag="acc0")
            acc1 = accp.tile([128, 128], f32, tag="acc1")
            # all loads issued up front on SP so the DMA engines stream
            # (the SP sequencer is in-order: a waiting instruction would
            # block later, dependency-free loads). Two batches per DMA to
            # halve the DMA instruction count (less sem-reuse relaying).
            xs = []
            for bp in range(BL // 2):
                # q = 2p + j: 6KB contiguous DRAM per partition
                x2 = xp.tile([128, 2, 2, D], f32)
                nc.sync.dma_start(
                    x2[:], cl[2 * bp:2 * bp + 2].rearrange(
                        "b (p j) d -> p b j d", j=2))
                xs.append(x2)
            for bp in range(BL // 2):
                # pass-through store (identity merge hot path) on the GpSimd
                # SWDGE: only waits on its own load
                nc.gpsimd.dma_start(
                    out[2 * bp:2 * bp + 2].rearrange("b (p j) d -> p b j d",
                                                     j=2), xs[bp][:])
            ytp4s = []
            for bp in range(BL // 2):
                x2 = xs[bp]
                # square (one big ACT op) + row-sums (one DVE reduce) per
                # batch of the pair
                sq = sqp.tile([128, 2, 2, D], f32)
                nc.scalar.activation(sq[:], x2[:],
                                     mybir.ActivationFunctionType.Square)
                s = normp.tile([128, 2, 2], f32, tag="s")
                nc.vector.reduce_sum(s[:], sq[:], axis=mybir.AxisListType.X)
                n = normp.tile([128, 2, 2], f32, tag="n")
                nc.scalar.sqrt(n[:], s[:])
                nc.vector.tensor_scalar_max(n[:], n[:], EPS)
                inv = normp.tile([128, 2, 2], f32, tag="inv")
                nc.vector.reciprocal(inv[:], n[:])

                # y_pair dims (j, b, d) so each j-slice is one contiguous
                # [128, 768] u16 transpose input covering BOTH batches
                y = yp.tile([128, 2, 2, D], fp8)
                for b2 in range(2):
                    for j in range(2):
                        nc.vector.tensor_scalar_mul(
                            y[:, j, b2, :], x2[:, b2, j, :],
                            inv[:, b2, j:j + 1])
                # uint16 view: each element is an fp8 (d=2t, d=2t+1) pair;
                # the xbar transpose moves pairs; chunk (b, m) gets columns
                # c<128 -> q=2c (even), c>=128 -> q=2(c-128)+1 (odd)
                ytp4 = ytp.tile([128, 2, KC2, Q], u16)
                yto = ytp4.rearrange("p b m c -> p (b m) c")
                # high priority: the transposes gate TensorE; don't let the
                # static scheduler slot later pairs' squares ahead of them
                with tc.high_priority():
                    for j in range(2):
                        nc.scalar.dma_start_transpose(
                            yto[:, :, j * 128:(j + 1) * 128],
                            y[:, j].bitcast(u16))
                ytp4s.append(ytp4)
            for b in range(BL):
                ytf8 = ytp4s[b // 2][:, b % 2].bitcast(fp8)
                # [128, KC2, 512] = (q, r) pairs

                # sim is symmetric: even rows x all cols + odd x odd.
                # fp8 DoubleRowSwInterleave: stationary reads the interleaved
                # pairs directly (contiguous), with columns reversed by the
                # hardware; moving operand uses the [2, N] strided view.
                # Contraction (p, r) <-> d is a bijection either way.
                swi = mybir.MatmulPerfMode.DoubleRowSwInterleave
                ps0 = ps0p.tile([128, Q], f32)
                ps1 = ps1p.tile([128, 128], f32)
                for k in range(KC2):
                    v = ytf8[:, k, :].rearrange("p (q r) -> p r q", r=2)
                    nc.tensor.matmul(ps0[:], ytf8[:, k, 0:256], v[:],
                                     start=(k == 0), stop=(k == KC2 - 1),
                                     perf_mode=swi)
                for k in range(KC2):
                    v = ytf8[:, k, :].rearrange("p (q r) -> p r q", r=2)
                    nc.tensor.matmul(ps1[:], ytf8[:, k, 256:512],
                                     v[:, :, 128:256],
                                     start=(k == 0), stop=(k == KC2 - 1),
                                     perf_mode=swi)
                if b == 0:
                    nc.vector.tensor_copy(acc0[:], ps0[:])
                    nc.vector.tensor_copy(acc1[:], ps1[:])
                else:
                    nc.vector.tensor_max(acc0[:], acc0[:], ps0[:])
                    nc.vector.tensor_max(acc1[:], acc1[:], ps1[:])
            nc.sync.dma_start(msim[0:128, :], acc0[:])
            nc.sync.dma_start(msim[128:256, 128:256], acc1[:])
    nc.compile()
    return nc


def _get_nc():
    global _nc_cache
    if _nc_cache is None:
        _nc_cache = _build()
    return _nc_cache


# maxsim column position n <-> cluster index q (even block, then odd block)
_COLQ = np.concatenate([np.arange(0, Q, 2), np.arange(1, Q, 2)])
# row position m <-> q: SwInterleave reverses stationary columns per block
_ROWQ = np.concatenate([2 * (127 - np.arange(128)),
                        2 * (127 - np.arange(128)) + 1])


def run_device(clusters, **spmd_kwargs):
    """Shard over 8 cores, run, gather. Returns (out [B,Q,D], maxsim [Q,Q]
    in true q order, BassKernelResults)."""
    clusters = np.ascontiguousarray(clusters, dtype=np.float32)
    assert clusters.shape == (B, Q, D), clusters.shape
    in_maps = [
        {"clusters": np.ascontiguousarray(clusters[i * BL:(i + 1) * BL])}
        for i in range(NCORES)
    ]
    res = run_bass_kernel_spmd(_get_nc(), in_maps,
                               core_ids=list(range(NCORES)), **spmd_kwargs)
    out = np.concatenate([res.results[i]["out"] for i in range(NCORES)], axis=0)
    mp = np.max(np.stack([res.results[i]["maxsim"] for i in range(NCORES)]),
                axis=0)
    msim = np.zeros((Q, Q), np.float32)
    msim[np.ix_(_ROWQ, _COLQ)] = mp
    msim = np.maximum(msim, msim.T)
    return out, msim, res


def _host_collapse(clusters):
    """Exact f32 replication of the reference (rare path: only when some
    pair is near/above the similarity threshold)."""
    norm = np.maximum(
        np.sqrt((clusters.astype(np.float32) ** 2).sum(-1, keepdims=True)), EPS
    )
    ncl = clusters / norm
    pair = np.zeros((Q, Q), dtype=bool)
    for b in range(B):
        pair |= (ncl[b] @ ncl[b].T) > THRESHOLD
    c = clusters.copy()
    iu, ju = np.triu_indices(Q, k=1)
    for i, j in zip(iu, ju):
        if pair[i, j]:
            ni = (c[:, i] + c[:, j]) * np.float32(0.5)
            c[:, i] = ni
            c[:, j] = ni
    return c


def kernel(clusters):
    clusters = np.ascontiguousarray(clusters, dtype=np.float32)
    out, msim, _ = run_device(clusters)
    iu, ju = np.triu_indices(Q, k=1)
    if np.nanmax(msim[iu, ju]) > DEVICE_TRIGGER:
        return _host_collapse(clusters)
    return out
